# revision 1
# baseline (speedup 1.0000x reference)
"""Trainium2 Bass kernel for nn_Coref span scorer (T=20000, widths 1..10).

Strategy (per core, 8-way token-sharded, 2560-token slabs with halo):
  Feature-major on-chip layout. Softmax-pool normalization handled by the
  relu positive-homogeneity gauge trick: every span column is scaled by
  Z = sum_window exp(logit) > 0, so
      q = relu(Z*(A + B_sh) + Cp),  Cp = sum_j shift(F', j)
      r = relu(W2^T q [+ Z x b2])
      sp = w3^T r ;   true score = sp / Z + b3   (host-side divide)
  with F' = (W1c^T embeds^T + b1) * exp(logits) and A/B the W1a/W1b state
  projections computed ONCE and reused by all 10 widths via shifted slices.
"""
import os
import sys

sys.path.insert(0, "/opt/trn_rl_repo")

import numpy as np
import ml_dtypes

BF16 = ml_dtypes.bfloat16

T = 20000
NCORES = 8
L = 2500          # owned span starts per core
SLAB = 2560       # token slab (covers spans + width-10 halo, 20*128)
PADW = 16
EXT = SLAB + PADW
NW = 10
SD = 400          # state dim
ED = 300          # embed dim
HID = 150
CH = 512
NCH = SLAB // CH

SD_CHUNKS = [(0, 128), (128, 256), (256, 384), (384, 400)]
ED_CHUNKS = [(0, 128), (128, 256), (256, 300)]
H_CHUNKS = [(0, 128), (128, 150)]   # K-chunks of a 150-dim hidden
M_HALVES = [(0, 128), (128, 150)]   # M-halves of a 150-dim output

_CACHE = {}


def _build_program(use_b2):
    from contextlib import ExitStack
    import concourse.bacc as bacc
    import concourse.tile as tile
    from concourse import mybir

    f32 = mybir.dt.float32
    b16 = mybir.dt.bfloat16
    AF = mybir.ActivationFunctionType
    ALU = mybir.AluOpType

    nc = bacc.Bacc("TRN2", target_bir_lowering=False, debug=False)

    d_sT = nc.dram_tensor("sT", [SD, SLAB], b16, kind="ExternalInput")
    d_eT = nc.dram_tensor("eT", [ED, SLAB], b16, kind="ExternalInput")
    d_aw1 = nc.dram_tensor("aw1", [SD, HID], b16, kind="ExternalInput")
    d_aw2 = nc.dram_tensor("aw2", [HID, HID], b16, kind="ExternalInput")
    d_aw3 = nc.dram_tensor("aw3", [HID, 1], b16, kind="ExternalInput")
    d_ab1 = nc.dram_tensor("ab1", [HID, 1], f32, kind="ExternalInput")
    d_ab2 = nc.dram_tensor("ab2", [HID, 1], f32, kind="ExternalInput")
    d_ab3 = nc.dram_tensor("ab3", [1, 1], f32, kind="ExternalInput")
    d_w1 = nc.dram_tensor("w1", [1100, HID], b16, kind="ExternalInput")
    d_sb1 = nc.dram_tensor("sb1", [HID, 1], b16, kind="ExternalInput")
    d_sb2 = nc.dram_tensor("sb2", [1, HID], b16, kind="ExternalInput")
    d_w2 = nc.dram_tensor("w2", [HID, HID], b16, kind="ExternalInput")
    d_w3 = nc.dram_tensor("w3", [HID, 1], b16, kind="ExternalInput")

    d_spo = nc.dram_tensor("sp_out", [NW, SLAB], f32, kind="ExternalOutput")
    d_zo = nc.dram_tensor("z_out", [NW, SLAB], b16, kind="ExternalOutput")

    with tile.TileContext(nc) as tc, ExitStack() as ctx:
        wp = ctx.enter_context(tc.tile_pool(name="wp", bufs=1))
        bp = ctx.enter_context(tc.tile_pool(name="bp", bufs=1))
        kp = ctx.enter_context(tc.tile_pool(name="kp", bufs=1))
        pp = ctx.enter_context(tc.tile_pool(name="pp", bufs=2, space="PSUM"))

        def load(pool, dram, shape, dtype, tag, src=None):
            t = pool.tile(shape, dtype, tag=tag)
            nc.sync.dma_start(t[:, :], src if src is not None else dram[:, :])
            return t

        # --- weights to SBUF ---
        sT = [load(wp, d_sT, [k1 - k0, SLAB], b16, f"sT{i}", d_sT[k0:k1, :])
              for i, (k0, k1) in enumerate(SD_CHUNKS)]
        eT = [load(wp, d_eT, [k1 - k0, SLAB], b16, f"eT{i}", d_eT[k0:k1, :])
              for i, (k0, k1) in enumerate(ED_CHUNKS)]
        aw1 = [load(wp, d_aw1, [k1 - k0, HID], b16, f"aw1{i}", d_aw1[k0:k1, :])
               for i, (k0, k1) in enumerate(SD_CHUNKS)]
        aw2 = [load(wp, d_aw2, [k1 - k0, HID], b16, f"aw2{i}", d_aw2[k0:k1, :])
               for i, (k0, k1) in enumerate(H_CHUNKS)]
        aw3 = [load(wp, d_aw3, [k1 - k0, 1], b16, f"aw3{i}", d_aw3[k0:k1, :])
               for i, (k0, k1) in enumerate(H_CHUNKS)]
        w1a = [load(wp, d_w1, [k1 - k0, HID], b16, f"w1a{i}", d_w1[k0:k1, :])
               for i, (k0, k1) in enumerate(SD_CHUNKS)]
        w1b = [load(wp, d_w1, [k1 - k0, HID], b16, f"w1b{i}", d_w1[400 + k0:400 + k1, :])
               for i, (k0, k1) in enumerate(SD_CHUNKS)]
        w1c = [load(wp, d_w1, [k1 - k0, HID], b16, f"w1c{i}", d_w1[800 + k0:800 + k1, :])
               for i, (k0, k1) in enumerate(ED_CHUNKS)]
        w2 = [load(wp, d_w2, [k1 - k0, HID], b16, f"w2{i}", d_w2[k0:k1, :])
              for i, (k0, k1) in enumerate(H_CHUNKS)]
        w3 = [load(wp, d_w3, [k1 - k0, 1], b16, f"w3{i}", d_w3[k0:k1, :])
              for i, (k0, k1) in enumerate(H_CHUNKS)]
        ab1 = [load(wp, d_ab1, [m1 - m0, 1], f32, f"ab1{i}", d_ab1[m0:m1, :])
               for i, (m0, m1) in enumerate(M_HALVES)]
        ab2 = [load(wp, d_ab2, [m1 - m0, 1], f32, f"ab2{i}", d_ab2[m0:m1, :])
               for i, (m0, m1) in enumerate(M_HALVES)]
        ab3 = load(wp, d_ab3, [1, 1], f32, "ab3")
        sb1 = [load(wp, d_sb1, [m1 - m0, 1], b16, f"sb1{i}", d_sb1[m0:m1, :])
               for i, (m0, m1) in enumerate(M_HALVES)]
        sb2 = load(wp, d_sb2, [1, HID], b16, "sb2")
        ones = wp.tile([1, 128], b16, name="ones", tag="ones")
        nc.vector.memset(ones[:, :], 1.0)

        def mlp(dsts, rhs, wts, biases, func, outw=SLAB):
            """dsts: list per M-half of SBUF dest tiles [mh, outw];
            rhs: list per K-chunk of SBUF tiles [kc, SLAB];
            wts: list per K-chunk of weight tiles [kc, M_total]."""
            nchunks = outw // CH
            for c in range(nchunks):
                cs = slice(c * CH, (c + 1) * CH)
                for h, (m0, m1) in enumerate(M_HALVES[:len(dsts)]):
                    ps = pp.tile([m1 - m0, CH], f32, name=f"ps{m1 - m0}", tag=f"ps{m1 - m0}")
                    for ki in range(len(wts)):
                        nc.tensor.matmul(
                            ps[:, :], wts[ki][:, m0:m1], rhs[ki][:, cs],
                            start=(ki == 0), stop=(ki == len(wts) - 1))
                    bias = biases[h] if biases is not None else 0.0
                    nc.scalar.activation(dsts[h][:, cs], ps[:, :], func, bias=bias)

        # --- attention MLP (feature-major) ---
        ha = [bp.tile([128, SLAB], b16, name="ha0", tag="ha0"), bp.tile([22, SLAB], b16, name="ha1", tag="ha1")]
        mlp(ha, sT, aw1, [ab1[0][:, :], ab1[1][:, :]], AF.Relu)
        hb = [bp.tile([128, SLAB], b16, name="hb0", tag="hb0"), bp.tile([22, SLAB], b16, name="hb1", tag="hb1")]
        mlp(hb, ha, aw2, [ab2[0][:, :], ab2[1][:, :]], AF.Relu)
        e16 = bp.tile([1, EXT], b16, name="e16", tag="e16")
        nc.vector.memset(e16[:, SLAB:EXT], 0.0)
        for c in range(NCH):
            cs = slice(c * CH, (c + 1) * CH)
            ps = pp.tile([1, CH], f32, name="ps1", tag="ps1")
            for ki in range(2):
                nc.tensor.matmul(ps[:, :], aw3[ki][:, 0:1], hb[ki][:, cs],
                                 start=(ki == 0), stop=(ki == 1))
            nc.scalar.activation(e16[0:1, cs], ps[:, :], AF.Exp, bias=ab3[0:1, :])

        # --- broadcast e to 128 partitions ---
        ebc = bp.tile([128, EXT], b16, name="ebc", tag="ebc")
        nc.vector.memset(ebc[:, SLAB:EXT], 0.0)
        for c in range(NCH):
            cs = slice(c * CH, (c + 1) * CH)
            ps = pp.tile([128, CH], f32, name="ps128", tag="ps128")
            nc.tensor.matmul(ps[:, :], ones[0:1, :], e16[0:1, cs], start=True, stop=True)
            nc.scalar.activation(ebc[:, cs], ps[:, :], AF.Copy)

        # --- shared projections A, B, F' ---
        A = [bp.tile([128, SLAB], b16, name="A0", tag="A0"), bp.tile([22, SLAB], b16, name="A1", tag="A1")]
        mlp(A, sT, w1a, None, AF.Copy)
        B = [bp.tile([128, EXT], b16, name="B0", tag="B0"), bp.tile([22, EXT], b16, name="B1", tag="B1")]
        nc.vector.memset(B[0][:, SLAB:EXT], 0.0)
        nc.vector.memset(B[1][:, SLAB:EXT], 0.0)
        mlp(B, sT, w1b, None, AF.Copy)
        F0 = [bp.tile([128, SLAB], b16, name="F00", tag="F00"), bp.tile([22, SLAB], b16, name="F01", tag="F01")]
        mlp(F0, eT, w1c, None, AF.Copy)
        Fp = [bp.tile([128, EXT], b16, name="Fp0", tag="Fp0"), bp.tile([22, EXT], b16, name="Fp1", tag="Fp1")]
        nc.vector.memset(Fp[0][:, SLAB:EXT], 0.0)
        nc.vector.memset(Fp[1][:, SLAB:EXT], 0.0)
        for h, (m0, m1) in enumerate(M_HALVES):
            nc.vector.scalar_tensor_tensor(
                Fp[h][:, 0:SLAB], F0[h][:, 0:SLAB], sb1[h][:, :],
                ebc[0:m1 - m0, 0:SLAB], op0=ALU.add, op1=ALU.mult)

        # --- per-width loop ---
        ZB = bp.tile([128, SLAB], b16, name="ZB", tag="ZB")
        Cp = [bp.tile([128, SLAB], b16, name="Cp0", tag="Cp0"), bp.tile([22, SLAB], b16, name="Cp1", tag="Cp1")]

        for n in range(1, NW + 1):
            sh = n - 1
            ss = slice(sh, sh + SLAB)
            if n == 1:
                nc.vector.tensor_copy(ZB[:, :], ebc[:, 0:SLAB])
                for h in range(2):
                    nc.vector.tensor_copy(Cp[h][:, :], Fp[h][:, 0:SLAB])
            else:
                nc.vector.tensor_add(ZB[:, :], ZB[:, :], ebc[:, ss])
                for h in range(2):
                    nc.vector.tensor_add(Cp[h][:, :], Cp[h][:, :], Fp[h][:, ss])
            q = [kp.tile([128, SLAB], b16, name="q0", tag="q0", bufs=2),
                 kp.tile([22, SLAB], b16, name="q1", tag="q1", bufs=2)]
            for h, (m0, m1) in enumerate(M_HALVES):
                mh = m1 - m0
                t = kp.tile([mh, SLAB], b16, name=f"t{h}", tag=f"t{h}", bufs=1)
                nc.gpsimd.tensor_add(t[:, :], A[h][:, :], B[h][:, ss])
                nc.vector.tensor_mul(t[:, :], t[:, :], ZB[0:mh, :])
                nc.vector.tensor_add(t[:, :], t[:, :], Cp[h][:, :])
                nc.scalar.activation(q[h][:, :], t[:, :], AF.Relu)
            # L2 + L3
            r = [kp.tile([128, SLAB], b16, name="r0", tag="r0", bufs=2), kp.tile([22, SLAB], b16, name="r1", tag="r1", bufs=2)]
            spn = kp.tile([1, SLAB], f32, name="spn", tag="spn", bufs=2)
            for c in range(NCH):
                cs = slice(c * CH, (c + 1) * CH)
                for h, (m0, m1) in enumerate(M_HALVES):
                    ps = pp.tile([m1 - m0, CH], f32, name=f"ps{m1 - m0}", tag=f"ps{m1 - m0}")
                    nc.tensor.matmul(ps[:, :], w2[0][:, m0:m1], q[0][:, cs],
                                     start=True, stop=False)
                    nc.tensor.matmul(ps[:, :], w2[1][:, m0:m1], q[1][:, cs],
                                     start=False, stop=not use_b2)
                    if use_b2:
                        nc.tensor.matmul(ps[:, :], sb2[0:1, m0:m1], ZB[0:1, cs],
                                         start=False, stop=True)
                    nc.scalar.activation(r[h][:, cs], ps[:, :], AF.Relu)
                ps3 = pp.tile([1, CH], f32, name="ps1", tag="ps1")
                nc.tensor.matmul(ps3[:, :], w3[0][:, 0:1], r[0][:, cs],
                                 start=True, stop=False)
                nc.tensor.matmul(ps3[:, :], w3[1][:, 0:1], r[1][:, cs],
                                 start=False, stop=True)
                nc.scalar.activation(spn[0:1, cs], ps3[:, :], AF.Copy)
            nc.sync.dma_start(d_zo[n - 1:n, :], ZB[0:1, :])
            nc.sync.dma_start(d_spo[n - 1:n, :], spn[0:1, :])
    nc.compile()
    return nc


def _get_program(use_b2):
    key = ("prog", use_b2)
    if key not in _CACHE:
        _CACHE[key] = _build_program(use_b2)
    return _CACHE[key]


def _prep_inputs(inputs):
    f32 = np.float32
    W = {k: np.asarray(v, f32) for k, v in inputs.items()}
    shared = {
        "aw1": W["attn_W1"].astype(BF16),
        "aw2": W["attn_W2"].astype(BF16),
        "aw3": W["attn_W3"].astype(BF16),
        "ab1": W["attn_b1"].reshape(HID, 1),
        "ab2": W["attn_b2"].reshape(HID, 1),
        "ab3": W["attn_b3"].reshape(1, 1),
        "w1": W["sc_W1"].astype(BF16),
        "sb1": W["sc_b1"].reshape(HID, 1).astype(BF16),
        "sb2": W["sc_b2"].reshape(1, HID).astype(BF16),
        "w2": W["sc_W2"].astype(BF16),
        "w3": W["sc_W3"].astype(BF16),
    }
    states, embeds = W["states"], W["embeds"]
    in_maps = []
    for d in range(NCORES):
        t0 = d * L
        hi = min(T, t0 + SLAB)
        ss = np.zeros((SLAB, SD), f32)
        se = np.zeros((SLAB, ED), f32)
        ss[:hi - t0] = states[t0:hi]
        se[:hi - t0] = embeds[t0:hi]
        m = dict(shared)
        m["sT"] = np.ascontiguousarray(ss.T).astype(BF16)
        m["eT"] = np.ascontiguousarray(se.T).astype(BF16)
        in_maps.append(m)
    return in_maps, float(W["sc_b3"][0])


_last_results = None


def kernel(**inputs):
    global _last_results
    from concourse.bass_utils import run_bass_kernel_spmd

    in_maps, b3 = _prep_inputs(inputs)
    use_b2 = bool(np.any(np.asarray(inputs["sc_b2"])))
    nc = _get_program(use_b2)
    res = run_bass_kernel_spmd(nc, in_maps, core_ids=list(range(NCORES)))
    _last_results = res
    outs = res.results
    parts = []
    for n in range(1, NW + 1):
        for d in range(NCORES):
            sp = np.asarray(outs[d]["sp_out"])[n - 1]
            z = np.asarray(outs[d]["z_out"])[n - 1].astype(np.float32)
            cnt = L if d < NCORES - 1 else L - (n - 1)
            parts.append(sp[:cnt] / z[:cnt] + b3)
    return np.concatenate(parts).astype(np.float32)

